# revision 39
# baseline (speedup 1.0000x reference)
# Self-contained Trainium2 Bass kernel for nn_AdaAttentionalGNN (B=2, D=256, H=4, N=M=2048, L=6).
# Sharding: data-parallel over batch B across 2 groups of 4 cores; within a group each core
# owns an N/4-column slice of the query axis. v2 restructure vs baseline:
#   - prune-prob accumulation stays in SBUF: e tiles for one head live in a 4-deep ring,
#     PE-transposed into PSUM after the head's Z is known, accumulated into a resident
#     fp16 acc (kills the 17MB-per-attn eT DRAM round trip).
#   - softmax Z normalization via direct row reciprocal + ones-matmul broadcast (no DRAM
#     bounce); per-partition reciprocal obtained by PE-transposing the reciprocal row.
#   - top-k masks built by bisection from SBUF acc at the end of the layer that produced
#     the acc, emitted as chunks drained through tail/allgather/next-layer slots.
#   - masks stored transposed as u8; v-bias folded into the merge bias on host; W1 bias
#     dropped (InstanceNorm cancels it exactly); bf16 delta all-gather with gpsimd
#     desc updates.
import sys
sys.path.insert(0, '/opt/trn_rl_repo')
import numpy as np

import concourse.bass as bass
import concourse.bacc as bacc
import concourse.tile as tile
import concourse.tile_utils as tile_utils
tile_utils.max_sbuf_usage = 208 * 1024
import concourse.mybir as mybir
from concourse.bass_utils import run_bass_kernel_spmd
from concourse import masks as masks_util

F32 = mybir.dt.float32
F32R = mybir.dt.float32r
BF16 = mybir.dt.bfloat16
FP16 = mybir.dt.float16
U8 = mybir.dt.uint8
I32 = mybir.dt.int32
ALU = mybir.AluOpType
ACTF = mybir.ActivationFunctionType
LN16 = float(np.log(16.0))

D = 256
H = 4
HD = 64
L = 6
NAMES = ("self", "cross", "self", "cross", "self", "cross")
POOLS = (1, 1, 2, 2, 2, 2)
EPS = 1e-5
BISECT = 11
NCORES = 8
GROUPS = [[0, 1, 2, 3], [4, 5, 6, 7]]
PAIRS = ('00', '11', '01', '10')


def head_perm():
    p = np.zeros(D, np.int64)
    for h in range(H):
        for hd in range(HD):
            p[h * HD + hd] = hd * H + h
    return p


def _pack_rows(a):
    C, X = a.shape
    n_hi = C // 128
    return np.ascontiguousarray(
        a.reshape(n_hi, 128, X).transpose(1, 0, 2).reshape(128, n_hi * X))


def _pack_bias(b):
    C = b.shape[0]
    return np.ascontiguousarray(b.reshape(C // 128, 128).T)


def build(NT, NL=L, DBG=False):
    SL = NT // 4            # this core's query slice
    MT = NT // 128          # 16 source m-tiles
    NH = SL // 128          # 4 query blocks of 128

    nc = bacc.Bacc("TRN2", target_bir_lowering=False, debug=False,
                   enable_asserts=False, num_devices=NCORES)

    d0_d = nc.dram_tensor("d0", [128, 2 * NT], F32R, kind="ExternalInput")
    d1_d = nc.dram_tensor("d1", [128, 2 * NT], F32R, kind="ExternalInput")
    wq_d = nc.dram_tensor("wq", [L, 128, 512], F32R, kind="ExternalInput")
    wk_d = nc.dram_tensor("wk", [L, 128, 512], F32R, kind="ExternalInput")
    wv_d = nc.dram_tensor("wv", [L, 128, 512], F32R, kind="ExternalInput")
    wm_d = nc.dram_tensor("wm", [L, 128, 512], BF16, kind="ExternalInput")
    w1a_d = nc.dram_tensor("w1a", [L, 128, 1024], F32R, kind="ExternalInput")
    w1b_d = nc.dram_tensor("w1b", [L, 128, 1024], BF16, kind="ExternalInput")
    w2_d = nc.dram_tensor("w2", [L, 128, 1024], BF16, kind="ExternalInput")
    bias_d = nc.dram_tensor("bias", [L, 128, 16], F32, kind="ExternalInput")
    out_d = nc.dram_tensor("out", [128, 4 * SL], F32, kind="ExternalOutput")

    desc = [nc.alloc_sbuf_tensor("desc0", [128, 2 * NT], F32R),
            nc.alloc_sbuf_tensor("desc1", [128, 2 * NT], F32R)]
    fl_sb = {p: nc.alloc_sbuf_tensor(f"fl_{p}", [128, NH], FP16) for p in PAIRS}
    sixt_bf = nc.alloc_sbuf_tensor("sixt_bf", [128, 512], BF16)
    ones128r = nc.alloc_sbuf_tensor("ones128r", [1, 128], F32R)
    ident = nc.alloc_sbuf_tensor("ident", [128, 128], BF16)

    ag_in = [nc.dram_tensor(f"ag_in{i}", [128, 4 * SL], BF16) for i in range(2)]
    ag_out = [nc.dram_tensor(f"ag_out{i}", [4 * 128, 4 * SL], BF16) for i in range(2)]
    ar_in = [nc.dram_tensor(f"ar_in{i}", [128, 8], F32) for i in range(4)]
    ar_out = [nc.dram_tensor(f"ar_out{i}", [128, 8], F32) for i in range(4)]
    rz_dram = nc.dram_tensor("rz_bounce", [1, SL], F32)

    kcnt = {p: NT for p in PAIRS}

    with tile.TileContext(nc) as tc:
        from contextlib import ExitStack
        ctx = ExitStack()
        wpool = ctx.enter_context(tc.tile_pool(name="wpool", bufs=1))
        kpool = ctx.enter_context(tc.tile_pool(name="kpool", bufs=1))
        vpool = ctx.enter_context(tc.tile_pool(name="vpool", bufs=1))
        qpool = ctx.enter_context(tc.tile_pool(name="qpool", bufs=1))
        epool = ctx.enter_context(tc.tile_pool(name="epool", bufs=4))
        ppool = ctx.enter_context(tc.tile_pool(name="ppool", bufs=1))
        mpool = ctx.enter_context(tc.tile_pool(name="mpool", bufs=1))
        zpool = ctx.enter_context(tc.tile_pool(name="zpool", bufs=1))
        ampool = ctx.enter_context(tc.tile_pool(name="ampool", bufs=1))
        dpool = ctx.enter_context(tc.tile_pool(name="dpool", bufs=2))
        gpool = ctx.enter_context(tc.tile_pool(name="gpool", bufs=2))
        spool = ctx.enter_context(tc.tile_pool(name="spool", bufs=2))
        psA = ctx.enter_context(tc.tile_pool(name="psA", bufs=2, space="PSUM"))
        psB = ctx.enter_context(tc.tile_pool(name="psB", bufs=2, space="PSUM"))
        psC = ctx.enter_context(tc.tile_pool(name="psC", bufs=2, space="PSUM"))
        psT = ctx.enter_context(tc.tile_pool(name="psT", bufs=1, space="PSUM"))

        nc.vector.memset(sixt_bf[:, :], 16.0)
        ln16 = spool.tile([128, 1], F32, tag="ln16", bufs=1)
        nc.vector.memset(ln16[:], LN16)
        onesf = spool.tile([1, 128], F32, tag="c128", bufs=1)
        nc.vector.memset(onesf[:], 1.0)
        nc.vector.tensor_copy(ones128r[:, :], onesf[:])
        masks_util.make_identity(nc, ident[:, :])

        nc.sync.dma_start(desc[0][:, :], d0_d[:, :])
        nc.sync.dma_start(desc[1][:, :], d1_d[:, :])

        pid = nc.vector.partition_id()
        off = (pid % 4) * SL

        def load_weights(l):
            w = {}
            for nm, dram, width, dt in (("wq", wq_d, 512, F32R), ("wk", wk_d, 512, F32R),
                                        ("wv", wv_d, 512, F32R), ("wm", wm_d, 512, BF16),
                                        ("w1a", w1a_d, 1024, F32R),
                                        ("w1b", w1b_d, 1024, BF16),
                                        ("w2", w2_d, 1024, BF16)):
                t = wpool.tile([128, width], dt, tag=nm, name=nm)
                nc.sync.dma_start(t[:], dram[l, :, :])
                w[nm] = t
            bt = wpool.tile([128, 16], F32, tag="bias")
            nc.sync.dma_start(bt[:], bias_d[l, :, :])
            w["bias"] = bt
            return w

        pacc_tiles = {}

        # ---- mask build as a list of chunks (closures) ---------------------
        def build_mask_chunks(pair, par, k, with_ties):
            """Bisection for the top-k threshold over the SBUF acc [q, m];
            emits maskT [m, q] u8 into res['maskT'] when chunks complete."""
            acc = pacc_tiles[pair]          # [128, NH*NT] fp16, query-major
            flt = fl_sb[pair]
            st = {}
            kf = float(k)
            res = {}

            def c_init():
                st['lo'] = spool.tile([128, NH], F32, tag=f"lo{par}", bufs=1, name=f"lo{par}")
                st['hi'] = spool.tile([128, NH], F32, tag=f"hi{par}", bufs=1, name=f"hi{par}")
                st['cntlo'] = spool.tile([128, NH], F32, tag=f"cl{par}", bufs=1, name=f"cl{par}")
                st['mid'] = spool.tile([128, NH], F32, tag=f"md{par}", bufs=1, name=f"md{par}")
                st['cm'] = spool.tile([128, NH], F32, tag=f"cc{par}", bufs=1, name=f"cc{par}")
                st['ge'] = spool.tile([128, NH], I32, tag=f"ge{par}", bufs=1, name=f"ge{par}")
                st['gei'] = spool.tile([128, NH], I32, tag=f"gi{par}", bufs=1, name=f"gi{par}")
                st['scr'] = ampool.tile([128, NT], U8, tag="scr", name=f"scr{par}", bufs=2)
                nc.vector.memset(st['hi'][:], 4.0 / k)
                st['negt'] = spool.tile([128, NH], F32, tag=f"ng{par}", bufs=1, name=f"ng{par}")
                st['sgn'] = spool.tile([128, NH], F32, tag=f"sg{par}", bufs=1, name=f"sg{par}")
                if with_ties:
                    st['flf'] = spool.tile([128, NH], F32, tag=f"ff{par}", bufs=1, name=f"ff{par}")
                    nc.vector.tensor_copy(st['flf'][:], flt[:, :])
                    nc.vector.tensor_copy(st['lo'][:], st['flf'][:])
                    nc.vector.tensor_scalar(st['negt'][:], st['flf'][:],
                                            -(1.0 + 2.0 ** -12), None, ALU.mult)
                    for nh in range(2):
                        nc.vector.tensor_scalar(
                            st['scr'][:], acc[:, nh * NT:(nh + 1) * NT],
                            st['flf'][:, nh:nh + 1], 0.0, ALU.is_gt, ALU.add,
                            accum_out=st['cntlo'][:, nh:nh + 1])
                    for nh in (2, 3):
                        sgscr = ampool.tile([128, NT], FP16, tag=f"ct", name=f"sg{par}")
                        nc.scalar.activation(sgscr[:], acc[:, nh * NT:(nh + 1) * NT],
                                             ACTF.Sign, bias=st['negt'][:, nh:nh + 1],
                                             accum_out=st['sgn'][:, nh:nh + 1])
                    nc.vector.tensor_scalar(st['cntlo'][:, 2:4], st['sgn'][:, 2:4],
                                            0.5, float(NT) / 2, ALU.mult, ALU.add)
                else:
                    nc.vector.memset(st['lo'][:], 0.0)
                    nc.vector.memset(st['cntlo'][:], float(NT))

            def c_iter():
                nc.vector.tensor_add(st['mid'][:], st['lo'][:], st['hi'][:])
                nc.vector.tensor_scalar_mul(st['mid'][:], st['mid'][:], 0.5)
                nc.vector.tensor_scalar(st['negt'][:], st['mid'][:],
                                        -(1.0 + 2.0 ** -12), None, ALU.mult)
                for nh in range(2):
                    nc.vector.tensor_scalar(
                        st['scr'][:], acc[:, nh * NT:(nh + 1) * NT],
                        st['mid'][:, nh:nh + 1], 0.0, ALU.is_gt, ALU.add,
                        accum_out=st['cm'][:, nh:nh + 1])
                for nh in (2, 3):
                    sgscr = ampool.tile([128, NT], FP16, tag=f"ct", name=f"sgi{par}")
                    nc.scalar.activation(sgscr[:], acc[:, nh * NT:(nh + 1) * NT],
                                         ACTF.Sign, bias=st['negt'][:, nh:nh + 1],
                                         accum_out=st['sgn'][:, nh:nh + 1])
                nc.vector.tensor_scalar(st['cm'][:, 2:4], st['sgn'][:, 2:4],
                                        0.5, float(NT) / 2, ALU.mult, ALU.add)
                nc.vector.tensor_scalar(st['ge'][:], st['cm'][:], kf, None, ALU.is_ge)
                nc.vector.tensor_scalar(st['gei'][:], st['cm'][:], kf, None, ALU.is_lt)
                nc.vector.copy_predicated(st['lo'][:], st['ge'][:], st['mid'][:])
                nc.vector.copy_predicated(st['cntlo'][:], st['ge'][:], st['cm'][:])
                nc.vector.copy_predicated(st['hi'][:], st['gei'][:], st['mid'][:])

            def c_pre_fin():
                if with_ties:
                    st['r'] = spool.tile([128, NH], F32, tag=f"rf{par}", bufs=1, name=f"rf{par}")
                    nc.vector.tensor_scalar(st['r'][:], st['cntlo'][:], -1.0, kf,
                                            ALU.mult, ALU.add)
                st['maskT'] = mpool.tile([128, MT * SL], U8, tag=f"mT{pair}", name=f"mT{pair}")
                res['maskT'] = st['maskT']

            def make_fin(nh):
                def c_fin():
                    a_nh = acc[:, nh * NT:(nh + 1) * NT]
                    minv = ampool.tile([128, NT], BF16, tag="ct", name=f"ct{par}")
                    if with_ties:
                        tiet = ampool.tile([128, NT], U8, tag="tt", name=f"tt{par}")
                        cumt = ampool.tile([128, NT], FP16, tag="cu", name=f"cu{par}")
                        nc.vector.tensor_scalar(minv[:], a_nh,
                                                st['lo'][:, nh:nh + 1], None, ALU.is_le)
                        nc.vector.tensor_scalar(tiet[:], a_nh,
                                                st['flf'][:, nh:nh + 1], None,
                                                ALU.is_equal)
                        nc.vector.tensor_tensor_scan(cumt[:], tiet[:], tiet[:], 0.0,
                                                     ALU.add, ALU.bypass)
                        nc.vector.scalar_tensor_tensor(cumt[:], cumt[:],
                                                       st['r'][:, nh:nh + 1],
                                                       tiet[:], ALU.is_le, ALU.mult)
                        nc.vector.tensor_tensor(minv[:], minv[:], cumt[:], ALU.subtract)
                    else:
                        nc.vector.tensor_scalar(minv[:], a_nh,
                                                st['lo'][:, nh:nh + 1], None, ALU.is_le)
                    # transpose [q,m] -> [m,q] via PE; evict to u8 mask
                    mv = st['maskT'][:, :].rearrange("p (mt q) -> p mt q", mt=MT)
                    for mg in range(2):
                        tp = psC.tile([128, 1024], BF16, tag="cps", name=f"mtp{par}")
                        for mt_ in range(8):
                            mt = mg * 8 + mt_
                            nc.tensor.matmul(tp[:, mt_ * 128:(mt_ + 1) * 128],
                                             minv[:, mt * 128:(mt + 1) * 128],
                                             ident[:, :], is_transpose=True)
                        for mt_ in range(8):
                            mt = mg * 8 + mt_
                            nc.vector.tensor_copy(
                                mv[:, mt, nh * 128:(nh + 1) * 128],
                                tp[:, mt_ * 128:(mt_ + 1) * 128])
                return c_fin

            chunks = [c_init] + [c_iter] * BISECT + [c_pre_fin] + \
                     [make_fin(nh) for nh in range(NH)]
            return chunks, res

        side_work = []

        def slot():
            if side_work:
                side_work.pop(0)()

        def drain():
            while side_work:
                side_work.pop(0)()

        # ---- attention propagation ----------------------------------------
        def attn_prop(l, w, pair, ti, si, maskT, feeds, last):
            dt_, ds_ = desc[ti], desc[si]
            bias = w["bias"]
            # xsl: this core's query-column slice of the target desc
            xsl = qpool.tile([128, 2 * SL], F32R, tag="xsl")
            for chc in range(2):
                nc.vector.tensor_copy(xsl[:, chc * SL:(chc + 1) * SL],
                                      dt_[:, bass.ds(off + chc * NT, SL)])
            q = qpool.tile([128, 2 * SL], F32R, tag="q")
            for mt_ in range(2):
                qp = psC.tile([128, 512], F32, tag="cps")
                for kt in range(2):
                    nc.tensor.matmul(qp[:, 0:SL],
                                     w["wq"][:, kt * 256 + mt_ * 128: kt * 256 + mt_ * 128 + 128],
                                     xsl[:, kt * SL:(kt + 1) * SL],
                                     start=(kt == 0), stop=(kt == 1))
                nc.scalar.activation(q[:, mt_ * SL:(mt_ + 1) * SL], qp[:, 0:SL],
                                     ACTF.Identity, bias=bias[:, 0 + mt_:1 + mt_])
            # k projection (full source)
            ksb = kpool.tile([128, 2 * NT], F32R, tag="k")
            for mt_ in range(2):
                for nt in range(NT // 512):
                    kp = psA.tile([128, 512], F32, tag="sps")
                    for kt in range(2):
                        nc.tensor.matmul(kp[:, 0:512],
                                         w["wk"][:, kt * 256 + mt_ * 128: kt * 256 + mt_ * 128 + 128],
                                         ds_[:, kt * NT + nt * 512: kt * NT + (nt + 1) * 512],
                                         start=(kt == 0), stop=(kt == 1))
                    nc.scalar.activation(ksb[:, mt_ * NT + nt * 512: mt_ * NT + (nt + 1) * 512],
                                         kp[:, 0:512], ACTF.Identity,
                                         bias=bias[:, 2 + mt_:3 + mt_])
            slot()
            # vT with interleaved ones columns (no bias: folded into merge bias)
            vT = vpool.tile([128, MT * 260], BF16, tag="vT")
            vview = vT[:, :].rearrange("p (mh c) -> p mh c", mh=MT)
            nc.vector.memset(vview[:, :, 64::65], 1.0)
            for mt_ in range(MT):
                vp = psC.tile([128, 512], F32, tag="cps")
                for kt in range(2):
                    nc.tensor.matmul(vp[:, 0:256],
                                     ds_[:, kt * NT + mt_ * 128: kt * NT + mt_ * 128 + 128],
                                     w["wv"][:, kt * 256:(kt + 1) * 256],
                                     start=(kt == 0), stop=(kt == 1))
                for h in range(H):
                    nc.vector.tensor_copy(vview[:, mt_, h * 65: h * 65 + 64],
                                          vp[:, h * 64:(h + 1) * 64])
            slot()
            if feeds:
                drain()     # acc tile reused below: all reads of the old
                            # generation must be emitted first
                pacc = ppool.tile([128, NH * NT], FP16, tag=f"pacc{ti}",
                                  name=f"pacc_{pair}_{l}")
                pacc_tiles[pair] = pacc
            # per-head attention; e kept in a 4-deep ring covering one head
            attall = zpool.tile([128, 2 * SL], BF16, tag="attall")
            flt_acc = None
            if feeds:
                flt_acc = spool.tile([128, NH], FP16, tag="fltacc", bufs=1)
            for h in range(H):
                po, chh = (h % 2) * 64, h // 2
                ap_ = psB.tile([128, SL], F32, tag="attps")
                egs = []
                for g in range(4):  # 4-mt groups
                    esb = epool.tile([128, 4 * SL], BF16, tag="esb")
                    egs.append(esb)
                    for mt_l in range(4):
                        mt_ = g * 4 + mt_l
                        sp = psA.tile([128, 512], F32, tag="sps")
                        nc.tensor.matmul(sp[:, 0:SL],
                                         ksb[po:po + 64, chh * NT + mt_ * 128: chh * NT + mt_ * 128 + 128],
                                         q[po:po + 64, chh * SL: chh * SL + SL],
                                         start=True, stop=True)
                        nc.scalar.activation(esb[:, mt_l * SL:(mt_l + 1) * SL],
                                             sp[:, 0:SL], ACTF.Exp,
                                             bias=ln16[:, 0:1])
                    if maskT is not None:
                        for mt_l in range(4):
                            nc.vector.copy_predicated(
                                esb[:, mt_l * SL:(mt_l + 1) * SL],
                                maskT[:, (g * 4 + mt_l) * SL:(g * 4 + mt_l + 1) * SL],
                                sixt_bf[:, 0:SL])
                    for mt_l in range(4):
                        mt_ = g * 4 + mt_l
                        nc.tensor.matmul(ap_[0:65, :],
                                         vT[:, mt_ * 260 + h * 65: mt_ * 260 + h * 65 + 65],
                                         esb[:, mt_l * SL:(mt_l + 1) * SL],
                                         start=(mt_ == 0), stop=(mt_ == MT - 1))
                # Z row -> [128, NH] via DRAM bounce -> partition-parallel
                # reciprocal -> bounce back for the ones-matmul broadcast
                zrow = spool.tile([1, SL], F32, tag="zzr", bufs=1, name="zrow")
                nc.scalar.activation(zrow[0:1, :], ap_[64:65, 0:SL], ACTF.Copy)
                nc.sync.dma_start(rz_dram[0:1, 0:SL], zrow[0:1, 0:SL])
                zr4 = spool.tile([128, NH], F32, tag="zr4", bufs=1)
                nc.sync.dma_start(
                    zr4[:, 0:NH],
                    rz_dram[0:1, 0:SL].rearrange("o (nh p) -> (o p) nh", p=128))
                r4h = spool.tile([128, NH], F32, tag=f"r4h{h}", bufs=1)
                nc.vector.reciprocal(r4h[:, :], zr4[:, :])
                nc.sync.dma_start(
                    rz_dram[0:1, 0:SL].rearrange("o (nh p) -> (o p) nh", p=128),
                    r4h[:, 0:NH])
                rrowr = spool.tile([1, SL], F32R, tag="zzr", bufs=1, name="rrowr")
                nc.gpsimd.dma_start(rrowr[0:1, 0:SL], rz_dram[0:1, 0:SL])
                rb = psC.tile([128, 512], F32, tag="cps")
                nc.tensor.matmul(rb[0:64, 0:SL], ones128r[0:1, 0:64],
                                 rrowr[0:1, 0:SL], start=True, stop=True)
                rbs = spool.tile([64, SL], F32, tag="rbs", bufs=1)
                nc.scalar.activation(rbs[:, :], rb[0:64, 0:SL], ACTF.Copy)
                nc.vector.tensor_tensor(attall[po:po + 64, chh * SL: chh * SL + SL],
                                        ap_[0:64, 0:SL], rbs[:], ALU.mult)
                if feeds:
                    # pacc += e^T * r4h  (PE transpose e -> PSUM, accumulate)
                    for g in range(4):
                        esb = egs[g]
                        for gg in range(2):
                            tp = psT.tile([128, 1024], BF16, tag="psTe", bufs=2)
                            for qb in range(NH):
                                for mt_l in range(2):
                                    mt_g = gg * 2 + mt_l
                                    nc.tensor.matmul(
                                        tp[:, qb * 256 + mt_l * 128: qb * 256 + mt_l * 128 + 128],
                                        esb[:, mt_g * SL + qb * 128: mt_g * SL + qb * 128 + 128],
                                        ident[:, :], is_transpose=True)
                            for qb in range(NH):
                                dst = pacc[:, qb * NT + g * 512 + gg * 256:
                                           qb * NT + g * 512 + gg * 256 + 256]
                                if h == 0:
                                    nc.vector.tensor_scalar(
                                        dst, tp[:, qb * 256:(qb + 1) * 256],
                                        r4h[:, qb:qb + 1], None, ALU.mult)
                                else:
                                    nc.vector.scalar_tensor_tensor(
                                        dst, tp[:, qb * 256:(qb + 1) * 256],
                                        r4h[:, qb:qb + 1], dst, ALU.mult, ALU.add)
                    # flt mirrors the pacc fp16 rounding path (x16 is exact)
                    if h == 0:
                        nc.vector.tensor_scalar_mul(flt_acc[:, :], r4h[:, :], 16.0)
                    else:
                        nc.vector.scalar_tensor_tensor(flt_acc[:, :], r4h[:, :],
                                                       16.0, flt_acc[:, :],
                                                       ALU.mult, ALU.add)
                slot()
            if feeds:
                nc.vector.tensor_copy(fl_sb[pair][:, :], flt_acc[:, :])
            # merge
            msg = ampool.tile([128, 2 * SL], BF16, tag="am", bufs=2)
            for mt_ in range(2):
                mp = psC.tile([128, 512], F32, tag="cps")
                for kt in range(2):
                    nc.tensor.matmul(mp[:, 0:SL],
                                     w["wm"][:, kt * 256 + mt_ * 128: kt * 256 + mt_ * 128 + 128],
                                     attall[:, kt * SL:(kt + 1) * SL],
                                     start=(kt == 0), stop=(kt == 1))
                nc.scalar.activation(msg[:, mt_ * SL:(mt_ + 1) * SL], mp[:, 0:SL],
                                     ACTF.Identity, bias=bias[:, 6 + mt_:7 + mt_])
            slot()
            # W1 + instnorm stats (no W1 bias: InstanceNorm cancels it)
            zsb = zpool.tile([128, 4 * SL], BF16, tag="zsb", bufs=2)
            hscr = zpool.tile([128, 4 * SL], BF16, tag="hsb")
            zsum = spool.tile([128, 4], F32, tag="zsum")
            zss = spool.tile([128, 4], F32, tag="zss")
            for mt_ in range(4):
                zp = psA.tile([128, 512], F32, tag="sps")
                for kt in range(4):
                    if kt < 2:
                        rhs = xsl[:, kt * SL:(kt + 1) * SL]
                        wt = w["w1a"][:, kt * 512 + mt_ * 128: kt * 512 + mt_ * 128 + 128]
                    else:
                        rhs = msg[:, (kt - 2) * SL:(kt - 1) * SL]
                        wt = w["w1b"][:, (kt - 2) * 512 + mt_ * 128: (kt - 2) * 512 + mt_ * 128 + 128]
                    nc.tensor.matmul(zp[:, 0:SL], wt, rhs,
                                     start=(kt == 0), stop=(kt == 3))
                nc.scalar.activation(zsb[:, mt_ * SL:(mt_ + 1) * SL], zp[:, 0:SL],
                                     ACTF.Identity, accum_out=zsum[:, mt_:mt_ + 1])
                nc.scalar.activation(hscr[:, mt_ * SL:(mt_ + 1) * SL],
                                     zsb[:, mt_ * SL:(mt_ + 1) * SL],
                                     ACTF.Square, accum_out=zss[:, mt_:mt_ + 1])
            sidx = (l % 2) * 2 + ti
            stpack = spool.tile([128, 8], F32, tag="stpack")
            nc.vector.tensor_copy(stpack[:, 0:4], zsum[:])
            nc.vector.tensor_copy(stpack[:, 4:8], zss[:])
            nc.sync.dma_start(ar_in[sidx][:, :], stpack[:])
            nc.gpsimd.collective_compute("AllReduce", ALU.add, replica_groups=GROUPS,
                                         ins=[ar_in[sidx][:, :].opt()],
                                         outs=[ar_out[sidx][:, :].opt()])
            stat = spool.tile([128, 8], F32, tag="stat")
            nc.sync.dma_start(stat[:], ar_out[sidx][:, :])
            slot()
            return dict(zsb=zsb, stat=stat)

        def attn_prop_tail(l, w, c, ti, last):
            bias = w["bias"]
            zsb, stat = c["zsb"], c["stat"]
            mu = spool.tile([128, 4], F32, tag="mu")
            var = spool.tile([128, 4], F32, tag="var")
            sd = spool.tile([128, 4], F32, tag="sd")
            rstd = spool.tile([128, 4], F32, tag="rstd")
            nbias = spool.tile([128, 4], F32, tag="nbias")
            tmp = spool.tile([128, 4], F32, tag="tmp4")
            inv_n = 1.0 / NT
            nc.vector.tensor_scalar_mul(mu[:], stat[:, 0:4], inv_n)
            nc.vector.tensor_scalar_mul(var[:], stat[:, 4:8], inv_n)
            nc.vector.tensor_tensor(tmp[:], mu[:], mu[:], ALU.mult)
            nc.vector.tensor_tensor(var[:], var[:], tmp[:], ALU.subtract)
            epst = spool.tile([128, 1], F32, tag="epst")
            nc.vector.memset(epst[:], EPS)
            nc.scalar.activation(sd[:], var[:], ACTF.Sqrt, bias=epst[:, 0:1])
            nc.vector.reciprocal(rstd[:], sd[:])
            nc.vector.tensor_tensor(nbias[:], mu[:], rstd[:], ALU.mult)
            nc.vector.tensor_scalar_mul(nbias[:], nbias[:], -1.0)
            hsb = zpool.tile([128, 4 * SL], BF16, tag="hsb")
            for mt_ in range(4):
                nc.scalar.activation(hsb[:, mt_ * SL:(mt_ + 1) * SL],
                                     zsb[:, mt_ * SL:(mt_ + 1) * SL],
                                     ACTF.Relu, bias=nbias[:, mt_:mt_ + 1],
                                     scale=rstd[:, mt_:mt_ + 1])
            slot()
            delta = dpool.tile([128, 2 * SL], BF16, tag="delta")
            for mt_ in range(2):
                dp = psA.tile([128, 512], F32, tag="sps")
                for kt in range(4):
                    nc.tensor.matmul(dp[:, 0:SL],
                                     w["w2"][:, kt * 256 + mt_ * 128: kt * 256 + mt_ * 128 + 128],
                                     hsb[:, kt * SL:(kt + 1) * SL],
                                     start=(kt == 0), stop=(kt == 3))
                nc.scalar.activation(delta[:, mt_ * SL:(mt_ + 1) * SL], dp[:, 0:SL],
                                     ACTF.Identity, bias=bias[:, 14 + mt_:15 + mt_])
            if last:
                # out = desc + delta, staged through the (dead) zsb slot
                osum = zpool.tile([128, 2 * SL], F32, tag="zsb", bufs=2)
                for ch in range(2):
                    nc.vector.tensor_tensor(osum[:, ch * SL:(ch + 1) * SL],
                                            delta[:, ch * SL:(ch + 1) * SL],
                                            desc[ti][:, bass.ds(off + ch * NT, SL)],
                                            ALU.add)
                    nc.sync.dma_start(out_d[:, (ti * 2 + ch) * SL:(ti * 2 + ch + 1) * SL],
                                      osum[:, ch * SL:(ch + 1) * SL])
            else:
                nc.sync.dma_start(ag_in[l % 2][:, ti * 2 * SL:(ti + 1) * 2 * SL],
                                  delta[:, :])
            slot()

        # ===== layers =====
        cur_masks = {}
        for l in range(NL):
            w = load_weights(l)
            pairs = [('01', 0, 1), ('10', 1, 0)] if NAMES[l] == 'cross' else \
                    [('00', 0, 0), ('11', 1, 1)]
            masked = POOLS[l] != 1
            feeds = l <= 3
            if masked:
                drain()   # this layer's masks must be fully built/emitted

            ctxs = {}
            for (pair, ti, si) in pairs:
                mres = cur_masks.get(pair)
                ctxs[pair] = attn_prop(l, w, pair, ti, si,
                                       mres['maskT'][:, :] if mres else None,
                                       feeds, l == NL - 1)

            # schedule mask builds for layer l+2 from this layer's acc
            if l + 2 < NL and POOLS[l + 2] != 1:
                with_ties = (l + 2) >= 4 and POOLS[l] != 1
                chunks = []
                results = []
                for i, (pair, ti, si) in enumerate(pairs):
                    ch, res = build_mask_chunks(pair, i,
                                                kcnt[pair] // POOLS[l + 2],
                                                with_ties)
                    kcnt[pair] //= POOLS[l + 2]
                    chunks.append(ch)
                    results.append((pair, res))
                inter = []
                for a, b in zip(chunks[0], chunks[1]):
                    inter.append(a)
                    inter.append(b)
                side_work.extend(inter)
                for pair, res in results:
                    cur_masks[pair] = res
            for (pair, ti, si) in pairs:
                attn_prop_tail(l, w, ctxs[pair], ti, l == NL - 1)
            if l < NL - 1:
                nc.gpsimd.collective_compute("AllGather", ALU.bypass,
                                             replica_groups=GROUPS,
                                             ins=[ag_in[l % 2][:, :].opt()],
                                             outs=[ag_out[l % 2][:, :].opt()])
                slot()
                for s in range(4):
                    for d_i in range(2):
                        agu = gpool.tile([128, 2 * SL], BF16, tag="agu")
                        nc.sync.dma_start(agu[:, :],
                                          ag_out[l % 2][s * 128:(s + 1) * 128,
                                                        d_i * 2 * SL:(d_i + 1) * 2 * SL])
                        for ch in range(2):
                            nc.gpsimd.tensor_tensor(
                                desc[d_i][:, ch * NT + s * SL: ch * NT + (s + 1) * SL],
                                desc[d_i][:, ch * NT + s * SL: ch * NT + (s + 1) * SL],
                                agu[:, ch * SL:(ch + 1) * SL],
                                ALU.add)
                    slot()
        drain()
        ctx.close()

    nc.compile()
    return nc


def prep_inputs(inputs, NT):
    import ml_dtypes
    bf16 = ml_dtypes.bfloat16
    perm = head_perm()
    iq = np.float32(1.0 / np.sqrt(HD))
    f32 = lambda a: np.asarray(a, np.float32)
    wq = np.stack([_pack_rows((f32(inputs['Wq'][l])[perm] * iq).T) for l in range(L)])
    wk = np.stack([_pack_rows(f32(inputs['Wk'][l])[perm].T) for l in range(L)])
    wv = np.stack([_pack_rows(f32(inputs['Wv'][l])[perm].T) for l in range(L)])
    wm = np.stack([_pack_rows(f32(inputs['Wm'][l]).T[perm]) for l in range(L)]).astype(bf16)
    w1 = np.stack([_pack_rows(f32(inputs['W1'][l]).T) for l in range(L)])
    w1a = np.ascontiguousarray(w1[:, :, 0:1024])
    w1b = np.ascontiguousarray(w1[:, :, 1024:2048]).astype(bf16)
    w2 = np.stack([_pack_rows(f32(inputs['W2'][l]).T) for l in range(L)]).astype(bf16)
    bias = np.zeros((L, 128, 16), np.float32)
    for l in range(L):
        bias[l, :, 0:2] = _pack_bias(f32(inputs['bq'][l])[perm] * iq)
        bias[l, :, 2:4] = _pack_bias(f32(inputs['bk'][l])[perm])
        # v bias folded into merge bias: prob sums to 1 over the source axis
        bmp = f32(inputs['Wm'][l]) @ f32(inputs['bv'][l])[perm] + f32(inputs['bm'][l])
        bias[l, :, 6:8] = _pack_bias(bmp)
        bias[l, :, 14:16] = _pack_bias(f32(inputs['b2'][l]))
    in_maps = []
    for c in range(NCORES):
        b = c // 4
        in_maps.append({
            "d0": _pack_rows(f32(inputs['desc0'][b])),
            "d1": _pack_rows(f32(inputs['desc1'][b])),
            "wq": wq, "wk": wk, "wv": wv, "wm": wm,
            "w1a": w1a, "w1b": w1b, "w2": w2,
            "bias": bias,
        })
    return in_maps


def assemble_out(results, NT):
    SL = NT // 4
    out = np.zeros((2, 2, D, NT), np.float32)
    for c in range(NCORES):
        b, s = c // 4, c % 4
        blob = results[c]["out"].reshape(128, 2, 2, SL)
        for d_i in range(2):
            for ch in range(2):
                out[d_i, b, ch * 128:(ch + 1) * 128, s * SL:(s + 1) * SL] = blob[:, d_i, ch]
    return out


_NC_CACHE = {}


def get_nc(NT, NL=L, DBG=False):
    key = (NT, NL, DBG)
    if key not in _NC_CACHE:
        _NC_CACHE[key] = build(NT, NL, DBG)
    return _NC_CACHE[key]


def run(inputs, NT, NL=L, DBG=False, **kw):
    nc = get_nc(NT, NL, DBG)
    in_maps = prep_inputs(inputs, NT)
    res = run_bass_kernel_spmd(nc, in_maps, core_ids=list(range(NCORES)), **kw)
    return assemble_out(res.results, NT), res


def kernel(**inputs):
    NT = int(np.asarray(inputs['desc0']).shape[2])
    out, _ = run(inputs, NT)
    return out
